# revision 20
# baseline (speedup 1.0000x reference)
"""Trainium2 Bass kernel for nn_Conv2dGeneral (capsule-style 4x4-pose conv).

Math (from the reference):
  out[b,o,X,Y,u,w] = sum_{cin,kx,ky,v} Wm[(cin,kx,ky),o,u,v] * x[b,cin,2X+kx,2Y+ky,4v+w] + bias[o]

Mapped to the PE array as a single 1152-deep contraction:
  K = (cin, v)  x  9 accumulation chunks over (kx, ky)   [9 x 128]
  M = (o, u)                                              [128 PSUM partitions]
  N = (X, Y, w)                                           [676 per batch image]

Data-parallel across 8 NeuronCores on the batch dim (8 images per core).

Host-side prep: x is re-laid-out to [(b), (cin,v), (r,c,w)] so each core's
shard DMAs as fully-contiguous partition lines; the stride-2 im2col window
gather then happens for free inside the matmul moving-operand access
pattern (no patch materialization, each x byte read once from HBM).

Overlap schedule (the finish time is max_b(arrival_b - 2.7us*b) + PE time,
so the arrival offsets of x images 0/1 and the PE stream are the critical
path; framework preamble ~7.2us and teardown ~8us are fixed):
  - PE warms its HAM clock gate on a DVE-memset tile starting right after
    the framework preamble (no DMA dependency), so the 1.2->2.4 GHz ramp
    is done before the first real matmul.
  - The Sync HWDGE ring carries ONLY the 8 x-image DMAs; W and bias ride
    the ACT ring concurrently (SDMA engines round-robin rings per packet),
    landing every image ~0.7us earlier than a shared ring would.
  - Output staged fp16 (error << gate; fp32 accumulation in PSUM), shipped
    per-image from the then-idle Sync queue, gated on act_sem; the last
    image computes as three groups (N=364/208/104) and ships in three
    pieces so the post-last-matmul eviction + DMA chain is short.
"""

import numpy as np

B, CIN, COUT = 64, 32, 32
KK, STRIDE = 3, 2
WIN, HH = 28, 16
H = 4
WOUT = (WIN - KK) // STRIDE + 1  # 13
NCORES = 8
BPC = B // NCORES                # batches per core
RWIN = 2 * (WOUT - 1) + KK       # 27: x row 27 is never read (stride 2)
RCW = RWIN * WIN * H             # 3024 free elements per (cin,v) partition.
                                 # Rows trimmed to 27 (3.6% less x traffic);
                                 # COLUMNS stay 28 so the row stride remains
                                 # 224B = 7x32B -- a 27-col slice breaks the
                                 # 32B stride phase of the PE moving-operand
                                 # reads and costs ~14% matmul cadence.
NOUT = WOUT * WOUT * H           # 676 outputs per (o,u) partition per image
XSPLIT = ((0, 7), (7, 6))        # two PSUM groups: X rows [0,7) and [7,13)
HNOUT = 7 * WOUT * H             # 364: first-half (X<7) slice of an image
# (b, X0, nX, psum_bank, pe_sem_inc): images 0-6 use two groups (7+6 X-rows)
# with one batched pe_sem inc at image end; image 7 uses three groups
# (7+4+2 rows, banks 6/7/0) with per-group incs so the tail eviction chain
# after the last matmul is short (final ACT is N=104, final DMA 26KB).
GROUPS = []
for _b in range(7):
    GROUPS.append((_b, 0, 7, (2 * _b) % 8, 0))
    GROUPS.append((_b, 7, 6, (2 * _b + 1) % 8, 2))
GROUPS += [(7, 0, 7, 6, 1), (7, 7, 4, 7, 1), (7, 11, 2, 0, 1)]
# PENDING[gi]: groups whose eviction is unlocked by the inc at group gi
PENDING = {}
_acc = []
for _gi, _g in enumerate(GROUPS):
    _acc.append(_g)
    if _g[4]:
        PENDING[_gi] = _acc
        _acc = []
WARMUP = 64                      # PE warmup matmuls: engines unblock ~7.6us,
                                 # x0 gates ~13.2us; cold matmuls run 107ns
                                 # and the HAM flips to 56ns after ~3.4us

_cache = {}


def _build_bass():
    """Raw-bass build (no Tile): this toolchain's walrus codegen allows only
    ONE sync-wait per instruction, so all cross-engine sync is explicit
    single-sem waits; ordering beyond that rides on hardware transitivity.

    Engines: SP triggers the input DMAs, DVE memsets the warmup tile, PE
    runs 16 accumulation groups of 9 matmuls (one per kernel tap), ACT
    evicts PSUM->SBUF (adding bias, casting to fp16) and ships each image.
    """
    import concourse.bass as bass
    import concourse.mybir as mybir

    f32 = mybir.dt.float32
    f16 = mybir.dt.float16
    NG = 2 * BPC              # 16 PSUM accumulation groups

    nc = bass.Bass()
    x_d = nc.declare_dram_parameter("x", [BPC, 128, RCW], f16, isOutput=False)
    w_d = nc.declare_dram_parameter("w", [128, 9 * 128], f16, isOutput=False)
    b_d = nc.declare_dram_parameter("b", [128, 1], f32, isOutput=False)
    o_d = nc.declare_dram_parameter("out", [BPC, 128, NOUT], f16, isOutput=True)

    from contextlib import ExitStack

    with ExitStack() as ctx:
        wt = ctx.enter_context(nc.sbuf_tensor([128, 9 * 128], f16))
        bt = ctx.enter_context(nc.sbuf_tensor([128, 1], f32))
        wu = ctx.enter_context(nc.sbuf_tensor([128, 128], f16))
        gt = ctx.enter_context(nc.sbuf_tensor([128, BPC, RCW], f16))
        ot = ctx.enter_context(nc.sbuf_tensor([128, BPC, NOUT], f16))
        ps = ctx.enter_context(nc.psum_tensor([128, 8, 512], f32))
        wt_sem = ctx.enter_context(nc.semaphore("wt_sem"))
        bias_sem = ctx.enter_context(nc.semaphore("bias_sem"))
        wu_sem = ctx.enter_context(nc.semaphore("wu_sem"))
        g_sems = [
            ctx.enter_context(nc.semaphore(f"g_sem{b}")) for b in range(BPC)
        ]
        pe_sem = ctx.enter_context(nc.semaphore("pe_sem"))
        act_sem = ctx.enter_context(nc.semaphore("act_sem"))
        out_sem = ctx.enter_context(nc.semaphore("out_sem"))
        block = ctx.enter_context(nc.Block())
        wtr = wt[:, :].rearrange("p (k m) -> p k m", k=9)

        @block.vector
        def _(vector):
            vector.memset(wu[:, :], 0.0).then_inc(wu_sem, 1)

        @block.sync
        def _(sync):
            # The Sync ring carries ONLY the x images: W/bias ride the ACT
            # ring and stream concurrently (SDMA engines round-robin the two
            # HWDGE rings per packet), so every image's bytes land ~0.7us
            # earlier and the ring's completion backlog stays short. The
            # finish time is max_b(arrival_b - 2.7us*b) + PE time, so the
            # arrival offsets of images 0 and 1 are the critical path.
            for b in range(0, BPC):
                sync.dma_start(gt[:, b, :], x_d[b]).then_inc(g_sems[b], 16)
            # Output shipping also lives here: the Sync queue is idle once
            # the inputs are issued, while on the ACT queue each ~0.6us
            # DMA-issue slot would delay subsequent evictions. Image 7 ships
            # in three pieces so the final (latency-critical) transfer is
            # only 26KB.
            for b in range(BPC - 1):
                sync.wait_ge(act_sem, 2 * b + 2)
                sync.dma_start(o_d[b], ot[:, b, :]).then_inc(out_sem, 16)
            O2 = 11 * WOUT * H  # 572
            sync.wait_ge(act_sem, 15)
            sync.dma_start(o_d[7, :, :HNOUT], ot[:, 7, :HNOUT]).then_inc(
                out_sem, 16
            )
            sync.wait_ge(act_sem, 16)
            sync.dma_start(o_d[7, :, HNOUT:O2], ot[:, 7, HNOUT:O2]).then_inc(
                out_sem, 16
            )
            sync.wait_ge(act_sem, 17)
            sync.dma_start(o_d[7, :, O2:], ot[:, 7, O2:]).then_inc(out_sem, 16)
            sync.wait_ge(out_sem, 16 * (BPC + 2))

        @block.tensor
        def _(tensor):
            # Warm the PE HAM clock gate (cold = 1.2 GHz) on the zeroed DVE
            # tile while W/x stream in; ~32 cold matmuls flip it to 2.4 GHz,
            # the rest keep it busy until the first x chunk lands.
            tensor.wait_ge(wu_sem, 1)
            for i in range(WARMUP):
                tensor.matmul(
                    ps[:, 7, :128], wu[:, :], wu[:, :], start=True, stop=True
                )
            tensor.wait_ge(wt_sem, 16)

            def win(b, X0, nX, kk):
                kx, ky = divmod(kk, 3)
                gr = gt[:, b, :].rearrange(
                    "p (r c w) -> p r c w", r=RWIN, c=WIN
                )
                return gr[
                    :,
                    2 * X0 + kx : 2 * X0 + kx + 2 * nX - 1 : 2,
                    ky : ky + 2 * WOUT - 1 : 2,
                    :,
                ]

            # Images 0-6 run TAP-MAJOR over their two PSUM groups: the two
            # matmuls of a tap share the stationary operand, and an MM whose
            # weights equal the previous MM's skips the weight-swap bubble
            # (measured: identical-weight warmups hit N/2.4+2.5 exactly,
            # changing-weight MMs pay ~5ns more).
            for b in range(7):
                tensor.wait_ge(g_sems[b], 16)
                if b >= 4:
                    tensor.wait_ge(act_sem, 2 * (b - 4) + 2)
                bk0, bk1 = (2 * b) % 8, (2 * b + 1) % 8
                for kk in range(9):
                    tensor.matmul(
                        ps[:, bk0, : 7 * WOUT * H],
                        wtr[:, kk, :],
                        win(b, 0, 7, kk),
                        start=(kk == 0),
                        stop=(kk == 8),
                    )
                    mm = tensor.matmul(
                        ps[:, bk1, : 6 * WOUT * H],
                        wtr[:, kk, :],
                        win(b, 7, 6, kk),
                        start=(kk == 0),
                        stop=(kk == 8),
                    )
                # batched: one serialized sem-write per image (~26ns each)
                mm.then_inc(pe_sem, 2)
            # Image 7 stays group-major (three groups, per-group incs) so
            # its tail evictions pipeline with the remaining matmuls.
            tensor.wait_ge(g_sems[7], 16)
            # image 7 spans banks 6,7,0 -> needs groups 6,7,8 drained
            tensor.wait_ge(act_sem, 9)
            for X0, nX, bank in ((0, 7, 6), (7, 4, 7), (11, 2, 0)):
                for kk in range(9):
                    mm = tensor.matmul(
                        ps[:, bank, : nX * WOUT * H],
                        wtr[:, kk, :],
                        win(7, X0, nX, kk),
                        start=(kk == 0),
                        stop=(kk == 8),
                    )
                mm.then_inc(pe_sem, 1)

        @block.scalar
        def _(scalar):
            # W and bias ride the ACT HWDGE ring, concurrent with the x
            # stream on the Sync ring. W is first: the first real
            # LDWEIGHTS gates on it.
            scalar.dma_start(wt[:, :], w_d[:, :]).then_inc(wt_sem, 16)
            scalar.dma_start(bt[:, :], b_d[:, :]).then_inc(bias_sem, 16)
            scalar.wait_ge(bias_sem, 16)
            need = 0
            for gi, (b, X0, nX, bank, inc) in enumerate(GROUPS):
                need += inc
                if inc:
                    scalar.wait_ge(pe_sem, need)
                    # evict every group finished by this inc point
                    for b2, X02, nX2, bank2, _ in PENDING[gi]:
                        off = X02 * WOUT * H
                        scalar.activation(
                            ot[:, b2, off : off + nX2 * WOUT * H],
                            ps[:, bank2, : nX2 * WOUT * H],
                            mybir.ActivationFunctionType.Identity,
                            bias=bt[:, :],
                        ).then_inc(act_sem, 1)

    return nc


def _prep_inputs(x, W, bias):
    # x: (B, CIN, 28, 28, 16) -> xp[b, cin*4+v, (r*28+c)*4+w] = x[b,cin,r,c,4v+w]
    # for r < 27 (row 27 is outside every stride-2 3x3 window; columns keep
    # all 28 entries purely for the 32B stride alignment of the PE reads).
    # fp16: PE runs fp32 matmuls as LOW_HIGH double passes; fp16 is single-pass
    # with fast-weight-load, and halves the dominant HBM traffic. Max rel err
    # ~3e-4 at this contraction depth (fp32 PSUM accumulation).
    xp = np.ascontiguousarray(
        x.reshape(B, CIN, WIN, WIN, H, H)[:, :, :RWIN].transpose(
            0, 1, 4, 2, 3, 5
        )
    ).reshape(B, CIN * H, RCW).astype(np.float16)
    # W: (1, 288, 32, 1, 1, 4, 4); p = cin*9 + kx*3 + ky
    # wt_sb[cin*4+v, kk*128 + o*4+u] = Wm[cin*9+kk, o, u, v]
    Wm = np.asarray(W, dtype=np.float32).reshape(CIN, KK * KK, COUT, H, H)
    wt_sb = np.ascontiguousarray(
        Wm.transpose(0, 4, 1, 2, 3)  # cin, v, kk, o, u
    ).reshape(128, 9 * 128).astype(np.float16)
    bias_v = np.ascontiguousarray(
        np.repeat(np.asarray(bias, dtype=np.float32).reshape(COUT), H)
    ).reshape(128, 1)
    return xp, wt_sb, bias_v


def _shard_x(xp, core):
    # per-core input: [BPC, 128, RCW] fp16
    return np.ascontiguousarray(xp[core * BPC : (core + 1) * BPC])


def _unprep_output(full):
    # full: (B, 128, NOUT) with partition o*4+u, free (X, Y, w)
    out = (
        full.astype(np.float32)
        .reshape(B, COUT, H, WOUT, WOUT, H)
        .transpose(0, 1, 3, 4, 2, 5)
        .reshape(B, COUT, WOUT, WOUT, HH)
    )
    return np.ascontiguousarray(out)


def run_device(in_maps, trace=False, tmpdir=None):
    from concourse.bass_utils import run_bass_kernel_spmd

    if "nc" not in _cache:
        _cache["nc"] = _build_bass()
    return run_bass_kernel_spmd(
        _cache["nc"], in_maps, list(range(NCORES)), trace=trace, tmpdir=tmpdir
    )


def kernel(x, W, bias):
    x = np.asarray(x, dtype=np.float32)
    xp, wt_sb, bias_v = _prep_inputs(x, W, bias)
    in_maps = [
        {"x": _shard_x(xp, i), "w": wt_sb, "b": bias_v} for i in range(NCORES)
    ]
    res = run_device(in_maps, trace=False)
    full = np.concatenate(
        [res.results[i]["out"] for i in range(NCORES)], axis=0
    )
    return _unprep_output(full)
